# revision 12
# baseline (speedup 1.0000x reference)
"""Trainium2 Bass kernel for nn_AttnBlock_79517024518715.

Computes (per batch element b):
  w        = weight_norm(conv_v, conv_g)                    (folded on host)
  y        = causal_conv1d(x, w, K=3) + conv_b              (PE matmuls)
  yglu     = glu(y)                   [512, 256]            (ACT sigmoid + DVE)
  hT       = (fc1_w @ yglu) + (word_embed + fc1_b).T        [512f, 256t]
  logits   = hT.T @ attn_feat                               [256t, 196s]
  score    = softmax(logits, axis=s)                        (output)
  ctxT     = attn_feat @ score.T                            [512f, 256t]
  outT     = fc2_wT.T @ ctxT + fc2_b                        [512c, 256t]
  out0     = outT + yglu + x                                (output)
Pass-through outputs word_embed / img_conv are returned host-side.

Sharding: pure data-parallel over batch, 8 batch elements per core on 8
NeuronCores, no collectives. Batch elements are processed in pairs so most
matmuls run with a 512-wide moving operand (full fp32r rate on the PE).
"""

import sys
from contextlib import ExitStack

sys.path.insert(0, "/opt/trn_rl_repo")

import numpy as np

import concourse.bacc as bacc
import concourse.mybir as mybir
import concourse.tile as tile
from concourse import bass_utils
from concourse.masks import make_identity

B, T, C_IN, C_OUT, KW = 64, 256, 512, 1024, 3
ATTN_CH = 512
HW_S = 196          # 14*14 attention spatial positions
S_PAD = 256         # padded spatial dim so score matmuls run at N=256
N_CORES = 8
B_PER = B // N_CORES      # 8 batch elements per core
NPAIR = B_PER // 2        # processed in pairs of 2

F32 = mybir.dt.float32
F32R = mybir.dt.float32r
AX_X = mybir.AxisListType.X
ALU = mybir.AluOpType
ACT_F = mybir.ActivationFunctionType

_CACHE = {}


def _mm(nc, out, lhsT, rhs, start, stop):
    nc.tensor.matmul(out, lhsT, rhs, start=start, stop=stop)


def _build(reps=None):
    """Build + compile the per-core Bass module (identical on all 8 cores).

    reps=None: the real kernel (external I/O). reps=N: timing variant — all
    real I/O lives in internal DRAM (nothing transferred) and the whole body
    runs N times in a hardware loop, so wall-clock differences between two
    reps values measure pure on-device execution time per iteration."""
    nc = bacc.Bacc("TRN2", target_bir_lowering=False, debug=False)

    timing = reps is not None
    kin = "Internal" if timing else "ExternalInput"
    kout = "Internal" if timing else "ExternalOutput"
    din = lambda n, s: nc.dram_tensor(n, s, F32, kind=kin).ap()
    dout = lambda n, s: nc.dram_tensor(n, s, F32, kind=kout).ap()
    if timing:
        dum_i = nc.dram_tensor("dum_i", [128, 4], F32, kind="ExternalInput").ap()
        dum_o = nc.dram_tensor("dum_o", [128, 4], F32, kind="ExternalOutput").ap()

    xp = din("xp", [NPAIR, 128, 4, 2, T + 2])     # padded x  [pair,p,cj,i,t]
    wef = din("wef", [NPAIR, 128, 4, 2, T])       # (we+fc1_b).T [pair,p,fj,i,t]
    af = din("af", [B_PER, 128, 4, HW_S])         # attn_feat  [b,p,fj,s] (f32)
    afT = din("afT", [B_PER, HW_S, ATTN_CH])      # attn_feat.T [b,s,f]
    wconv = din("wconv", [8, 128, 12, 128])       # [og, p, cj*3+k, o_in_g]
    wfc1 = din("wfc1", [128, 4, ATTN_CH])         # fc1_w.T  [p, cj, f]
    wfc2 = din("wfc2", [128, 4, C_OUT // 2])      # fc2_w.T  [p, fj, c]
    ba = din("ba", [128, 4])                      # conv_b[:512]  per c-chunk
    bg = din("bg", [128, 4])                      # -conv_b[512:]
    bo = din("bo", [128, 4])                      # fc2_b

    out0 = dout("out0", [B_PER, C_OUT // 2, T])
    score = dout("score", [B_PER, T, HW_S])

    with tile.TileContext(nc) as tc, ExitStack() as ctx:
        wp = ctx.enter_context(tc.tile_pool(name="wp", bufs=1))
        ip = ctx.enter_context(tc.tile_pool(name="ip", bufs=2))
        mp = ctx.enter_context(tc.tile_pool(name="mp", bufs=2))
        sp = ctx.enter_context(tc.tile_pool(name="sp", bufs=5))
        pp = ctx.enter_context(tc.tile_pool(name="pp", bufs=7, space="PSUM"))
        pt = ctx.enter_context(tc.tile_pool(name="pt", bufs=1, space="PSUM"))

        # ---- weights (DMA order matches first-use order; the HBM stream
        # is bandwidth-bound so order sets the PE start time) ----
        ba_sb = wp.tile([128, 4], F32)
        nc.sync.dma_start(out=ba_sb, in_=ba)
        bgn_sb = wp.tile([128, 4], F32)
        nc.sync.dma_start(out=bgn_sb, in_=bg)
        bo_sb = wp.tile([128, 4], F32)
        nc.sync.dma_start(out=bo_sb, in_=bo)
        wconv_sb = wp.tile([128, 12, C_OUT], F32R)
        ident = wp.tile([128, 128], F32)
        make_identity(nc, ident)
        zeros_sb = wp.tile([128, ATTN_CH], F32)
        nc.gpsimd.memset(zeros_sb, 0.0)
        wfc1_sb = wp.tile([128, 4, ATTN_CH], F32R)
        wfc2_sb = wp.tile([128, 4, C_OUT // 2], F32R)

        def load_wconv_group(og):
            nc.sync.dma_start(out=wconv_sb[:, :, og * 128:(og + 1) * 128],
                              in_=wconv[og].bitcast(F32R))

        if timing:
            dum_sb = wp.tile([128, 4], F32)
            nc.sync.dma_start(out=dum_sb, in_=dum_i)
            nc.sync.dma_start(out=dum_o, in_=dum_sb)
            loop_cm = tc.For_i(0, reps, 1, hint_engines=(
                mybir.EngineType.PE, mybir.EngineType.DVE,
                mybir.EngineType.Activation, mybir.EngineType.SP,
                mybir.EngineType.Pool))
            loop_cm.__enter__()

        for p in range(NPAIR):
            # ---- conv1d (causal, K=3) + GLU ----
            xp_sb = ip.tile([128, 4, 2, T + 2], F32R, tag="xp")
            for cj in range(4):
                nc.sync.dma_start(out=xp_sb[:, cj],
                                  in_=xp[p, :, cj].bitcast(F32R))
            yglu = mp.tile([128, 4, 2, T], F32R, tag="yglu")
            for j in range(4):
                if p == 0:
                    load_wconv_group(j)
                    load_wconv_group(j + 4)
                ps_a = pp.tile([128, 2, T], F32, tag="ps")
                ps_g = pp.tile([128, 2, T], F32, tag="ps")
                for n, psum in ((j, ps_a), (j + 4, ps_g)):
                    i = 0
                    for cj in range(4):
                        for k in range(KW):
                            _mm(nc, psum,
                                wconv_sb[:, cj * 3 + k, n * 128:(n + 1) * 128],
                                xp_sb[:, cj, :, k:k + T],
                                start=(i == 0), stop=(i == 11))
                            i += 1
                # e = exp(-(ps_g + conv_b_g)); sig = 1/(1+e)  (avoids a
                # second ACT table: only Exp is ever loaded)
                sig = sp.tile([128, 2, T], F32, tag="sig")
                nc.scalar.activation(sig, ps_g, ACT_F.Exp,
                                     bias=bgn_sb[:, j:j + 1], scale=-1.0)
                nc.vector.tensor_scalar_add(sig, sig, 1.0)
                nc.vector.reciprocal(sig, sig)
                # yglu = (ps_a + conv_b_a) * sig
                nc.vector.scalar_tensor_tensor(
                    yglu[:, j], ps_a, ba_sb[:, j:j + 1], sig,
                    op0=ALU.add, op1=ALU.mult)

            # ---- fc1: hT[f,t] = fc1_wT.T @ yglu + (we + fc1_b).T ----
            if p == 0:
                nc.sync.dma_start(out=wfc1_sb, in_=wfc1.bitcast(F32R))
            wef_sb = ip.tile([128, 4, 2, T], F32, tag="wef")
            nc.sync.dma_start(out=wef_sb, in_=wef[p])
            hsb = mp.tile([128, 4, 2, T], F32, tag="hsb")
            for fi in range(4):
                ps_h = pp.tile([128, 2, T], F32, tag="ps")
                for cj in range(4):
                    _mm(nc, ps_h, wfc1_sb[:, cj, fi * 128:(fi + 1) * 128],
                        yglu[:, cj], start=(cj == 0), stop=(cj == 3))
                nc.vector.tensor_add(hsb[:, fi], ps_h, wef_sb[:, fi])

            # ---- attention per batch element ----
            ctxT_sb = mp.tile([128, 4, 2, T], F32R, tag="ctxT")
            for i in range(2):
                b = 2 * p + i
                af_sb = ip.tile([128, 4, HW_S], F32, tag="af")
                nc.sync.dma_start(out=af_sb, in_=af[b])
                afT_sb = ip.tile([128, 2, ATTN_CH], F32R, tag="afT")
                nc.vector.tensor_copy(afT_sb[64:, 1], zeros_sb[64:])
                nc.sync.dma_start(out=afT_sb[:, 0], in_=afT[b, :128].bitcast(F32R))
                nc.sync.dma_start(out=afT_sb[:68, 1], in_=afT[b, 128:].bitcast(F32R))

                sc_sb = sp.tile([128, 2, S_PAD], F32, tag="sc")
                for m in range(2):
                    ps_sl = pp.tile([128, HW_S], F32, tag="ps")
                    for fi in range(4):
                        _mm(nc, ps_sl,
                            hsb[:, fi, i, m * 128:(m + 1) * 128],
                            af_sb[:, fi], start=(fi == 0), stop=(fi == 3))
                    # softmax over the 196 real spatial positions
                    mx = sp.tile([128, 1], F32, tag="mx")
                    nc.vector.reduce_max(mx, ps_sl, axis=AX_X,
                                         negate=True)
                    nc.gpsimd.memset(sc_sb[:, m, HW_S:], 0.0)
                    ssum = sp.tile([128, 1], F32, tag="ssum")
                    nc.scalar.activation(sc_sb[:, m, :HW_S], ps_sl,
                                         ACT_F.Exp, bias=mx, scale=1.0,
                                         accum_out=ssum)
                    rcp = sp.tile([128, 1], F32, tag="rcp")
                    nc.vector.reciprocal(rcp, ssum)
                    nc.vector.tensor_scalar_mul(sc_sb[:, m, :HW_S],
                                                sc_sb[:, m, :HW_S], rcp)
                    nc.scalar.dma_start(out=score[b, m * 128:(m + 1) * 128, :],
                                        in_=sc_sb[:, m, :HW_S])

                # transpose score -> [s, t] (PE transpose, 128x128 blocks)
                scT_sb = sp.tile([128, 2, T], F32R, tag="scT")
                for sj in range(2):
                    for m in range(2):
                        ps_t = pt.tile([128, 128], F32, tag="pst")
                        nc.tensor.transpose(
                            ps_t, sc_sb[:, m, sj * 128:(sj + 1) * 128], ident)
                        nc.vector.tensor_copy(
                            scT_sb[:, sj, m * 128:(m + 1) * 128], ps_t)

                # ctxT[f,t] = sum_s afT[s,f] * scoreT[s,t]
                for fi in range(4):
                    ps_c = pp.tile([128, T], F32, tag="ps")
                    for sj in range(2):
                        _mm(nc, ps_c,
                            afT_sb[:, sj, fi * 128:(fi + 1) * 128],
                            scT_sb[:, sj], start=(sj == 0), stop=(sj == 1))
                    nc.vector.tensor_copy(ctxT_sb[:, fi, i], ps_c)

            # ---- fc2 + bias + glu-residual + input residual ----
            if p == 0:
                nc.sync.dma_start(out=wfc2_sb, in_=wfc2.bitcast(F32R))
            for j in range(4):
                ps_o = pp.tile([128, 2, T], F32, tag="ps")
                for fi in range(4):
                    _mm(nc, ps_o, wfc2_sb[:, fi, j * 128:(j + 1) * 128],
                        ctxT_sb[:, fi], start=(fi == 0), stop=(fi == 3))
                t0 = sp.tile([128, 2, T], F32, tag="t0")
                # t0 = (ps_o + fc2_b) + yglu   (x residual is added on host)
                nc.vector.scalar_tensor_tensor(
                    t0, ps_o, bo_sb[:, j:j + 1], yglu[:, j],
                    op0=ALU.add, op1=ALU.add)
                for i in range(2):
                    nc.scalar.dma_start(
                        out=out0[2 * p + i, j * 128:(j + 1) * 128, :],
                        in_=t0[:, i, :])

        if timing:
            loop_cm.__exit__(None, None, None)

    nc.compile()
    return nc


def _prep_core(x, we2T, afp, aftp, core):
    """Per-core input map. x:[B,512,256] we2T:[B,512,256] afp:[B,128,4,256]
    aftp:[B,128,2,512]; core selects the B_PER-batch shard."""
    sl = slice(core * B_PER, (core + 1) * B_PER)
    xs = x[sl]
    xpad = np.zeros((B_PER, C_IN, T + 2), np.float32)
    xpad[:, :, 2:] = xs
    xp = np.ascontiguousarray(
        xpad.reshape(NPAIR, 2, 4, 128, T + 2).transpose(0, 3, 2, 1, 4))
    wef = np.ascontiguousarray(
        we2T[sl].reshape(NPAIR, 2, 4, 128, T).transpose(0, 3, 2, 1, 4))
    return {"xp": xp, "wef": wef, "af": afp[sl], "afT": aftp[sl]}


def _prep_inputs(x, word_embed, img_conv, conv_v, conv_g, conv_b,
                 fc1_w, fc1_b, fc2_w, fc2_b):
    """Full host-side prep: returns the per-core input maps."""
    # ---- host-side weight prep (identical for all cores) ----
    v = np.asarray(conv_v, np.float32)
    vnorm = np.sqrt(np.sum(v * v, axis=(1, 2), keepdims=True))
    w = (np.asarray(conv_g, np.float32)[:, None, None] * v / vnorm)
    wconv = (w.transpose(1, 2, 0).reshape(4, 128, KW, C_OUT)
             .transpose(1, 0, 2, 3).reshape(128, 4 * KW, 8, 128)
             .transpose(2, 0, 1, 3))
    wconv = np.ascontiguousarray(wconv)
    wfc1 = np.ascontiguousarray(
        np.asarray(fc1_w, np.float32).T.reshape(4, 128, ATTN_CH)
        .transpose(1, 0, 2))
    wfc2 = np.ascontiguousarray(
        np.asarray(fc2_w, np.float32).T.reshape(4, 128, C_OUT // 2)
        .transpose(1, 0, 2))
    cb = np.asarray(conv_b, np.float32)
    ba = np.ascontiguousarray(cb[:512].reshape(4, 128).T)
    bg = np.ascontiguousarray(-cb[512:].reshape(4, 128).T)
    bo = np.ascontiguousarray(np.asarray(fc2_b, np.float32).reshape(4, 128).T)
    wmap = {"wconv": wconv, "wfc1": wfc1, "wfc2": wfc2,
            "ba": ba, "bg": bg, "bo": bo}

    # ---- host-side activation prep ----
    we2T = np.ascontiguousarray(
        (word_embed + np.asarray(fc1_b, np.float32)).transpose(0, 2, 1))
    attn = img_conv.reshape(B, ATTN_CH, HW_S)
    afp = np.ascontiguousarray(
        attn.reshape(B, 4, 128, HW_S).transpose(0, 2, 1, 3))
    aftp = np.ascontiguousarray(attn.transpose(0, 2, 1))

    return [dict(_prep_core(x, we2T, afp, aftp, c), **wmap)
            for c in range(N_CORES)]


def kernel(x, word_embed, img_conv, prev_attn, conv_v, conv_g, conv_b,
           fc1_w, fc1_b, fc2_w, fc2_b):
    x = np.asarray(x, np.float32)
    word_embed = np.asarray(word_embed, np.float32)
    img_conv = np.asarray(img_conv, np.float32)

    if "nc" not in _CACHE:
        _CACHE["nc"] = _build()
    nc = _CACHE["nc"]

    in_maps = _prep_inputs(x, word_embed, img_conv, conv_v, conv_g, conv_b,
                           fc1_w, fc1_b, fc2_w, fc2_b)
    res = bass_utils.run_bass_kernel_spmd(nc, in_maps,
                                          core_ids=list(range(N_CORES)))

    out_full = np.empty((B, C_OUT // 2, T), np.float32)
    score_full = np.empty((B, T, HW_S), np.float32)
    for c in range(N_CORES):
        out_full[c * B_PER:(c + 1) * B_PER] = res.results[c]["out0"]
        score_full[c * B_PER:(c + 1) * B_PER] = res.results[c]["score"]
    out_full += x
    return out_full, word_embed, img_conv, score_full


# revision 29
# speedup vs baseline: 1.2250x; 1.2250x over previous
"""Trainium2 Bass kernel for nn_AttnBlock_79517024518715.

Computes (per batch element b):
  w        = weight_norm(conv_v, conv_g)                    (folded on host)
  y        = causal_conv1d(x, w, K=3) + conv_b              (PE matmuls)
  yglu     = glu(y)                   [512, 256]            (ACT sigmoid + DVE)
  hT       = (fc1_w @ yglu) + (word_embed + fc1_b).T        [512f, 256t]
  logits   = hT.T @ attn_feat                               [256t, 196s]
  score    = softmax(logits, axis=s)                        (output)
  ctxT     = attn_feat @ score.T                            [512f, 256t]
  outT     = fc2_wT.T @ ctxT + fc2_b                        [512c, 256t]
  out0     = outT + yglu + x                                (output)
Pass-through outputs word_embed / img_conv are returned host-side.

Sharding: pure data-parallel over batch, 8 batch elements per core on 8
NeuronCores, no collectives. Batch elements are processed in pairs so most
matmuls run with a 512-wide moving operand (full fp32r rate on the PE).
"""

import sys
from contextlib import ExitStack

sys.path.insert(0, "/opt/trn_rl_repo")

import numpy as np

import concourse.bacc as bacc
import concourse.mybir as mybir
import concourse.tile as tile
from concourse import bass_utils
from concourse.masks import make_identity

B, T, C_IN, C_OUT, KW = 64, 256, 512, 1024, 3
ATTN_CH = 512
HW_S = 196          # 14*14 attention spatial positions
S_PAD = 256         # padded spatial dim so score matmuls run at N=256
N_CORES = 8
B_PER = B // N_CORES      # 8 batch elements per core
NPAIR = B_PER // 2        # processed in pairs of 2

F32 = mybir.dt.float32
F32R = mybir.dt.float32r
AX_X = mybir.AxisListType.X
ALU = mybir.AluOpType
ACT_F = mybir.ActivationFunctionType

_CACHE = {}


def _mm(nc, out, lhsT, rhs, start, stop):
    nc.tensor.matmul(out, lhsT, rhs, start=start, stop=stop)


def _build(reps=None):
    """Build + compile the per-core Bass module (identical on all 8 cores).

    reps=None: the real kernel (external I/O). reps=N: timing variant — all
    real I/O lives in internal DRAM (nothing transferred) and the whole body
    runs N times in a hardware loop, so wall-clock differences between two
    reps values measure pure on-device execution time per iteration."""
    nc = bacc.Bacc("TRN2", target_bir_lowering=False, debug=False)

    timing = reps is not None
    kin = "Internal" if timing else "ExternalInput"
    kout = "Internal" if timing else "ExternalOutput"
    din = lambda n, s: nc.dram_tensor(n, s, F32, kind=kin).ap()
    dout = lambda n, s: nc.dram_tensor(n, s, F32, kind=kout).ap()
    if timing:
        dum_i = nc.dram_tensor("dum_i", [128, 4], F32, kind="ExternalInput").ap()
        dum_o = nc.dram_tensor("dum_o", [128, 4], F32, kind="ExternalOutput").ap()

    xp = din("xp", [NPAIR, 128, 4, 2, T + 2])     # padded x  [pair,p,cj,i,t]
    wef = din("wef", [NPAIR, 128, 4, 2, T])       # (we+fc1_b).T [pair,p,fj,i,t]
    af = din("af", [B_PER, 128, 4, HW_S])         # attn_feat  [b,p,fj,s] (f32)
    afT = din("afT", [B_PER, 2, 128, ATTN_CH])    # attn_feat.T [b,sj,s,f] pad
    wconv = din("wconv", [8, 128, 12, 128])       # [og, p, cj*3+k, o_in_g]
    wfc1 = din("wfc1", [128, 4, ATTN_CH])         # fc1_w.T  [p, cj, f]
    wfc2 = din("wfc2", [128, 4, C_OUT // 2])      # fc2_w.T  [p, fj, c]
    bias = din("bias", [128, 12])                 # [ba | -bg | fc2_b]

    out0 = dout("out0", [B_PER, C_OUT // 2, T])
    score = dout("score", [B_PER, T, HW_S])

    with tile.TileContext(nc) as tc, ExitStack() as ctx:
        wp = ctx.enter_context(tc.tile_pool(name="wp", bufs=1))
        ip = ctx.enter_context(tc.tile_pool(name="ip", bufs=2))
        mp = ctx.enter_context(tc.tile_pool(name="mp", bufs=2))
        sp = ctx.enter_context(tc.tile_pool(name="sp", bufs=4))
        pp = ctx.enter_context(tc.tile_pool(name="pp", bufs=5, space="PSUM"))
        pt = ctx.enter_context(tc.tile_pool(name="pt", bufs=3, space="PSUM"))

        # ---- weights (DMA order matches first-use order; the HBM stream
        # is bandwidth-bound so order sets the PE start time) ----
        bias_sb = wp.tile([128, 12], F32)
        ba_sb, bgn_sb, bo_sb = bias_sb[:, 0:4], bias_sb[:, 4:8], bias_sb[:, 8:12]
        wconv_sb = wp.tile([128, 12, C_OUT], F32R)
        ident = wp.tile([128, 128], F32)
        make_identity(nc, ident)
        wfc1_sb = wp.tile([128, 4, ATTN_CH], F32R)
        wfc2_sb = wp.tile([128, 4, C_OUT // 2], F32R)

        def load_wconv_group(og):
            nc.sync.dma_start(out=wconv_sb[:, :, og * 128:(og + 1) * 128],
                              in_=wconv[og].bitcast(F32R))

        if timing:
            dum_sb = wp.tile([128, 4], F32)
            nc.sync.dma_start(out=dum_sb, in_=dum_i)
            nc.sync.dma_start(out=dum_o, in_=dum_sb)
            loop_cm = tc.For_i(0, reps, 1, hint_engines=(
                mybir.EngineType.PE, mybir.EngineType.DVE,
                mybir.EngineType.Activation, mybir.EngineType.SP,
                mybir.EngineType.Pool))
            loop_cm.__enter__()

        for p in range(NPAIR):
            # ---- conv1d (causal, K=3) + GLU ----
            xp_sb = ip.tile([128, 4, 2, T + 2], F32R, tag="xp")
            if p == 0:
                nc.sync.dma_start(out=xp_sb[:, 0], in_=xp[p, :, 0].bitcast(F32R))
                load_wconv_group(0)
                load_wconv_group(4)
                nc.sync.dma_start(out=bias_sb, in_=bias)
                for cj in range(1, 4):
                    nc.sync.dma_start(out=xp_sb[:, cj],
                                      in_=xp[p, :, cj].bitcast(F32R))
            else:
                nc.sync.dma_start(out=xp_sb, in_=xp[p].bitcast(F32R))
            yglu = mp.tile([128, 4, 2, T], F32R, tag="yglu")
            for j in range(4):
                if p == 0 and j > 0:
                    load_wconv_group(j)
                    load_wconv_group(j + 4)
                ps_a = pp.tile([128, 2, T], F32, tag="ps")
                ps_g = pp.tile([128, 2, T], F32, tag="ps")
                for n, psum in ((j, ps_a), (j + 4, ps_g)):
                    i = 0
                    for cj in range(4):
                        for k in range(KW):
                            _mm(nc, psum,
                                wconv_sb[:, cj * 3 + k, n * 128:(n + 1) * 128],
                                xp_sb[:, cj, :, k:k + T],
                                start=(i == 0), stop=(i == 11))
                            i += 1
                # e = exp(-(ps_g + conv_b_g)); sig = 1/(1+e)  (avoids a
                # second ACT table: only Exp is ever loaded)
                sig = sp.tile([128, 2, T], F32, tag="sig")
                nc.scalar.activation(sig, ps_g, ACT_F.Exp,
                                     bias=bgn_sb[:, j:j + 1], scale=-1.0)
                nc.vector.tensor_scalar_add(sig, sig, 1.0)
                nc.vector.reciprocal(sig, sig)
                # yglu = (ps_a + conv_b_a) * sig
                nc.vector.scalar_tensor_tensor(
                    yglu[:, j], ps_a, ba_sb[:, j:j + 1], sig,
                    op0=ALU.add, op1=ALU.mult)

            # ---- fc1: hT[f,t] = fc1_wT.T @ yglu + (we + fc1_b).T ----
            if p == 0:
                nc.sync.dma_start(out=wfc1_sb, in_=wfc1.bitcast(F32R))
            wef_sb = ip.tile([128, 4, 2, T], F32, tag="wef")
            nc.sync.dma_start(out=wef_sb, in_=wef[p])
            hsb = mp.tile([128, 4, 2, T], F32, tag="hsb")
            for fi in range(4):
                ps_h = pp.tile([128, 2, T], F32, tag="ps")
                for cj in range(4):
                    _mm(nc, ps_h, wfc1_sb[:, cj, fi * 128:(fi + 1) * 128],
                        yglu[:, cj], start=(cj == 0), stop=(cj == 3))
                nc.vector.tensor_add(hsb[:, fi], ps_h, wef_sb[:, fi])

            # ---- attention: phase 1 = logits+softmax for both elements,
            # phase 2 = transpose+ctx; phases interleave across elements ----
            ctxT_sb = mp.tile([128, 4, 2, T], F32R, tag="ctxT")
            sc_sbs, afT_sbs = [], []
            for i in range(2):
                b = 2 * p + i
                af_sb = ip.tile([128, 4, HW_S], F32, tag="af")
                nc.sync.dma_start(out=af_sb, in_=af[b])
                afT_sb = ip.tile([128, 2, ATTN_CH], F32R, tag="afT")
                nc.sync.dma_start(
                    out=afT_sb,
                    in_=afT[b].rearrange("sj s f -> s sj f").bitcast(F32R))
                afT_sbs.append(afT_sb)

                sc_sb = sp.tile([128, 2, S_PAD], F32, tag="sc")
                sc_sbs.append(sc_sb)
                for m in range(2):
                    ps_sl = pp.tile([128, HW_S], F32, tag="ps")
                    for fi in range(4):
                        _mm(nc, ps_sl,
                            hsb[:, fi, i, m * 128:(m + 1) * 128],
                            af_sb[:, fi], start=(fi == 0), stop=(fi == 3))
                    # softmax over the 196 real spatial positions
                    mx = sp.tile([128, 1], F32, tag="mx")
                    nc.vector.reduce_max(mx, ps_sl, axis=AX_X,
                                         negate=True)
                    nc.gpsimd.memset(sc_sb[:, m, HW_S:], 0.0)
                    ssum = sp.tile([128, 1], F32, tag="ssum")
                    nc.scalar.activation(sc_sb[:, m, :HW_S], ps_sl,
                                         ACT_F.Exp, bias=mx, scale=1.0,
                                         accum_out=ssum)
                    rcp = sp.tile([128, 1], F32, tag="rcp")
                    nc.vector.reciprocal(rcp, ssum)
                    nc.vector.tensor_scalar_mul(sc_sb[:, m, :HW_S],
                                                sc_sb[:, m, :HW_S], rcp)
                nc.scalar.dma_start(
                    out=score[b].rearrange("(m t) s -> t m s", m=2),
                    in_=sc_sb[:, :, :HW_S])

            scT_sbs = []
            for i in range(2):
                sc_sb = sc_sbs[i]
                # transpose score -> [s, t] (PE transpose, 128x128 blocks)
                scT_sb = sp.tile([128, 2, T], F32R, tag="scT")
                scT_sbs.append(scT_sb)
                for sj in range(2):
                    for m in range(2):
                        ps_t = pt.tile([128, 128], F32, tag="pst")
                        nc.tensor.transpose(
                            ps_t, sc_sb[:, m, sj * 128:(sj + 1) * 128], ident)
                        nc.vector.tensor_copy(
                            scT_sb[:, sj, m * 128:(m + 1) * 128], ps_t)

            # ctxT[f,t] = sum_s afT[s,f] * scoreT[s,t]; fi-major so the fc2
            # accumulation (also fi-major) can start as soon as fi=0 lands
            for fi in range(4):
                for i in range(2):
                    ps_c = pp.tile([128, T], F32, tag="ps")
                    for sj in range(2):
                        _mm(nc, ps_c,
                            afT_sbs[i][:, sj, fi * 128:(fi + 1) * 128],
                            scT_sbs[i][:, sj], start=(sj == 0), stop=(sj == 1))
                    nc.scalar.copy(ctxT_sb[:, fi, i], ps_c)

            # ---- fc2 + bias + glu-residual + input residual ----
            if p == 0:
                nc.sync.dma_start(out=wfc2_sb, in_=wfc2.bitcast(F32R))
            for j in range(4):
                ps_o = pp.tile([128, 2, T], F32, tag="ps")
                for fi in range(4):
                    _mm(nc, ps_o, wfc2_sb[:, fi, j * 128:(j + 1) * 128],
                        ctxT_sb[:, fi], start=(fi == 0), stop=(fi == 3))
                t0 = sp.tile([128, 2, T], F32, tag="t0")
                # t0 = (ps_o + fc2_b) + yglu   (x residual is added on host)
                nc.vector.scalar_tensor_tensor(
                    t0, ps_o, bo_sb[:, j:j + 1], yglu[:, j],
                    op0=ALU.add, op1=ALU.add)
                nc.scalar.dma_start(
                    out=out0[2 * p:2 * p + 2, j * 128:(j + 1) * 128, :]
                    .rearrange("b c t -> c b t"),
                    in_=t0)

        if timing:
            loop_cm.__exit__(None, None, None)

    nc.compile()
    return nc


def _prep_core(x, we2T, afp, aftp, core):
    """Per-core input map. x:[B,512,256] we2T:[B,512,256] afp:[B,128,4,256]
    aftp:[B,128,2,512]; core selects the B_PER-batch shard."""
    sl = slice(core * B_PER, (core + 1) * B_PER)
    xs = x[sl]
    xpad = np.zeros((B_PER, C_IN, T + 2), np.float32)
    xpad[:, :, 2:] = xs
    xp = np.ascontiguousarray(
        xpad.reshape(NPAIR, 2, 4, 128, T + 2).transpose(0, 3, 2, 1, 4))
    wef = np.ascontiguousarray(
        we2T[sl].reshape(NPAIR, 2, 4, 128, T).transpose(0, 3, 2, 1, 4))
    return {"xp": xp, "wef": wef, "af": afp[sl], "afT": aftp[sl]}


def _prep_inputs(x, word_embed, img_conv, conv_v, conv_g, conv_b,
                 fc1_w, fc1_b, fc2_w, fc2_b):
    """Full host-side prep: returns the per-core input maps."""
    # ---- host-side weight prep (identical for all cores) ----
    v = np.asarray(conv_v, np.float32)
    vnorm = np.sqrt(np.sum(v * v, axis=(1, 2), keepdims=True))
    w = (np.asarray(conv_g, np.float32)[:, None, None] * v / vnorm)
    wconv = (w.transpose(1, 2, 0).reshape(4, 128, KW, C_OUT)
             .transpose(1, 0, 2, 3).reshape(128, 4 * KW, 8, 128)
             .transpose(2, 0, 1, 3))
    wconv = np.ascontiguousarray(wconv)
    wfc1 = np.ascontiguousarray(
        np.asarray(fc1_w, np.float32).T.reshape(4, 128, ATTN_CH)
        .transpose(1, 0, 2))
    wfc2 = np.ascontiguousarray(
        np.asarray(fc2_w, np.float32).T.reshape(4, 128, C_OUT // 2)
        .transpose(1, 0, 2))
    cb = np.asarray(conv_b, np.float32)
    bias = np.concatenate([cb[:512].reshape(4, 128).T,
                           -cb[512:].reshape(4, 128).T,
                           np.asarray(fc2_b, np.float32).reshape(4, 128).T],
                          axis=1)
    wmap = {"wconv": wconv, "wfc1": wfc1, "wfc2": wfc2,
            "bias": np.ascontiguousarray(bias)}

    # ---- host-side activation prep ----
    we2T = np.ascontiguousarray(
        (word_embed + np.asarray(fc1_b, np.float32)).transpose(0, 2, 1))
    attn = img_conv.reshape(B, ATTN_CH, HW_S)
    afp = np.ascontiguousarray(
        attn.reshape(B, 4, 128, HW_S).transpose(0, 2, 1, 3))
    aftp = np.zeros((B, 2, 128, ATTN_CH), np.float32)
    aftp.reshape(B, 256, ATTN_CH)[:, :HW_S] = attn.transpose(0, 2, 1)

    return [dict(_prep_core(x, we2T, afp, aftp, c), **wmap)
            for c in range(N_CORES)]


def kernel(x, word_embed, img_conv, prev_attn, conv_v, conv_g, conv_b,
           fc1_w, fc1_b, fc2_w, fc2_b):
    x = np.asarray(x, np.float32)
    word_embed = np.asarray(word_embed, np.float32)
    img_conv = np.asarray(img_conv, np.float32)

    if "nc" not in _CACHE:
        _CACHE["nc"] = _build()
    nc = _CACHE["nc"]

    in_maps = _prep_inputs(x, word_embed, img_conv, conv_v, conv_g, conv_b,
                           fc1_w, fc1_b, fc2_w, fc2_b)
    res = bass_utils.run_bass_kernel_spmd(nc, in_maps,
                                          core_ids=list(range(N_CORES)))

    out_full = np.empty((B, C_OUT // 2, T), np.float32)
    score_full = np.empty((B, T, HW_S), np.float32)
    for c in range(N_CORES):
        out_full[c * B_PER:(c + 1) * B_PER] = res.results[c]["out0"]
        score_full[c * B_PER:(c + 1) * B_PER] = res.results[c]["score"]
    out_full += x
    return out_full, word_embed, img_conv, score_full
